# revision 6
# baseline (speedup 1.0000x reference)
"""LIF spiking-neuron recurrence kernel for Trainium2 (Bass/Tile, 8-core SPMD).

Problem: x [32, 128, 32, 32, 8] f32, time on the LAST axis (T=8).
    u_0 = x_0;  o_t = (u_t > Vth);  u_{t+1} = TAU * u_t * (1 - o_t) + x_{t+1}
Output: spikes o [32, 128, 32, 32, 8] f32 (0.0 / 1.0).

Sharding: pure data-parallel over the batch dim (32 -> 4 per core, 8 cores),
no communication. Per core the shard is viewed as [512, 8192]: 512 partition
rows x (1024 pixels * 8 timesteps, t contiguous). Four SBUF tiles of
[128, 8192] per core; within a tile the per-timestep planes are the strided
views tile[:, t::8].

Per-timestep compute (on [128, 1024] views):
    m   = (u <= Vth)                  DVE tensor_scalar (is_le), dense, 2x f32
    o_t = 1 - m                       ACT activation(Copy, scale=-1, bias=1),
                                      written straight to the strided out view
    w   = (u * TAU) * m               DVE scalar_tensor_tensor (mult, mult)
    u   = w + x_{t+1}                 DVE tensor_tensor add (strided x read)

Multiplying by m in {0.0, 1.0} is exact, so results are bit-identical to the
reference ordering TAU*u*(1-o) + x.
"""

import numpy as np

import bass_rust
import concourse.bass as bass
import concourse.mybir as mybir
import concourse.tile as tile
from concourse.bass_utils import run_bass_kernel_spmd

VTH = 0.2
TAU = 0.25

N_CORES = 8
FULL_SHAPE = (32, 128, 32, 32, 8)
B_PER_CORE = FULL_SHAPE[0] // N_CORES  # 4
T = FULL_SHAPE[-1]  # 8

ROWS = 512  # per-core partition rows: 4*128*32*32*8 / 8192
FREE = 8192  # free dim per row: 1024 pixels * 8 timesteps
C = FREE // T  # 1024 pixels per partition row per tile
N_TILES = ROWS // 128  # 4

_cache: dict = {}


def _split_multi_waits(nc: bass.Bass) -> int:
    """Hoist all-but-one embedded sync waits onto standalone EventSemaphore
    instructions. The walrus build behind bass2jax rejects >1 sync wait per
    compute/DMA instruction ("Too many sync wait commands"); a standalone
    wait on the same engine stream immediately before is semantically
    identical."""
    n = 0
    for fn in nc.m.functions:
        for block in fn.blocks:
            out = []
            changed = False
            for ins in block.instructions:
                si = ins.sync_info
                waits = list(si.on_wait) if si is not None else []
                if len(waits) > 1:
                    for k, w in enumerate(waits[:-1]):
                        ev = mybir.InstEventSemaphore(
                            name=f"{ins.name}-hw{k}", ins=[], outs=[]
                        )
                        ev.sync_info = bass_rust.SyncInfo(
                            on_wait=[w], on_update=[]
                        )
                        ev.engine = ins.engine
                        nc.inst_map[ev.name] = ev
                        out.append(ev)
                        n += 1
                    si.on_wait = [waits[-1]]
                    changed = True
                out.append(ins)
            if changed:
                block.instructions = out
    return n


def _build_bass() -> bass.Bass:
    f32 = mybir.dt.float32
    Alu = mybir.AluOpType
    Act = mybir.ActivationFunctionType

    nc = bass.Bass(trn_type="TRN2")
    x_d = nc.dram_tensor("x", [ROWS, FREE], f32, kind="ExternalInput")
    y_d = nc.dram_tensor("y", [ROWS, FREE], f32, kind="ExternalOutput")

    with tile.TileContext(nc) as tc:
        with (
            tc.tile_pool(name="pin", bufs=2) as pin,
            tc.tile_pool(name="pout", bufs=2) as pout,
            tc.tile_pool(name="ptmp", bufs=2) as ptmp,
        ):
            for i in range(N_TILES):
                xt = pin.tile([128, FREE], f32, tag="xt")
                nc.sync.dma_start(xt, x_d[i * 128 : (i + 1) * 128, :])
                ot = pout.tile([128, FREE], f32, tag="ot")

                u = ptmp.tile([128, C], f32, tag="u")
                m = ptmp.tile([128, C], f32, tag="m")
                w = ptmp.tile([128, C], f32, tag="w")
                for t in range(T):
                    u_src = xt[:, 0::T] if t == 0 else u
                    # m = (u <= Vth) in {0.0, 1.0}
                    nc.vector.tensor_scalar(m, u_src, VTH, None, Alu.is_le)
                    # o_t = 1 - m, straight to the strided output plane
                    nc.scalar.activation(
                        ot[:, t::T], m, Act.Copy, bias=1.0, scale=-1.0
                    )
                    if t < T - 1:
                        # w = (u * TAU) * m
                        nc.vector.scalar_tensor_tensor(
                            w, u_src, TAU, m, Alu.mult, Alu.mult
                        )
                        # u = w + x_{t+1}
                        nc.vector.tensor_tensor(
                            u, w, xt[:, t + 1 :: T], Alu.add
                        )

                nc.sync.dma_start(y_d[i * 128 : (i + 1) * 128, :], ot)

    _split_multi_waits(nc)
    return nc


def kernel(x: np.ndarray) -> np.ndarray:
    assert x.shape == FULL_SHAPE, x.shape
    in_dtype = x.dtype

    if "nc" not in _cache:
        _cache["nc"] = _build_bass()
    nc = _cache["nc"]

    x = np.ascontiguousarray(x, dtype=np.float32)
    in_maps = [
        {"x": x[c * B_PER_CORE : (c + 1) * B_PER_CORE].reshape(ROWS, FREE)}
        for c in range(N_CORES)
    ]
    res = run_bass_kernel_spmd(nc, in_maps, core_ids=list(range(N_CORES)))
    out = np.concatenate(
        [
            res.results[c]["y"].reshape(B_PER_CORE, *FULL_SHAPE[1:])
            for c in range(N_CORES)
        ],
        axis=0,
    )
    return out.astype(in_dtype, copy=False)


# revision 9
# speedup vs baseline: 1.1397x; 1.1397x over previous
"""LIF spiking-neuron recurrence kernel for Trainium2 (Bass/Tile, 8-core SPMD).

Problem: x [32, 128, 32, 32, 8] f32, time on the LAST axis (T=8).
    u_0 = x_0;  o_t = (u_t > Vth);  u_{t+1} = TAU * u_t * (1 - o_t) + x_{t+1}
Output: spikes o [32, 128, 32, 32, 8] f32 (0.0 / 1.0).

Sharding: pure data-parallel over the batch dim (32 -> 4 per core, 8 cores),
no communication. While sharding, the host also lays each core's shard out
t-plane-major ([pixels, T] -> [T, pixels] per 1024-pixel row group) so every
on-chip operand is dense unit-stride; engines pay a ~2x throughput penalty on
strided (stride-8) access patterns, which the interleaved layout would force
on every timestep. The gather step inverts the layout on the way out.

Per-timestep compute (on [128, C] dense views):
    m   = (u <= Vth)                  DVE tensor_scalar (is_le), 2x f32 mode
    o_t = 1 - m                       ACT activation(Copy, scale=-1, bias=1)
    w   = (u * TAU) * m               DVE scalar_tensor_tensor (mult, mult)
    u   = w + x_{t+1}                 DVE tensor_tensor add

Multiplying by m in {0.0, 1.0} is exact, so results are bit-identical to the
reference ordering TAU*u*(1-o) + x.
"""

import numpy as np

import bass_rust
import concourse.bass as bass
import concourse.mybir as mybir
import concourse.tile as tile
from concourse.bass_utils import run_bass_kernel_spmd

VTH = 0.2
TAU = 0.25

N_CORES = 8
FULL_SHAPE = (32, 128, 32, 32, 8)
B_PER_CORE = FULL_SHAPE[0] // N_CORES  # 4
T = FULL_SHAPE[-1]  # 8

ROWS = 512  # per-core partition rows: 4*128*32*32*8 / 8192
FREE = 8192  # free dim per row
C = FREE // T  # 1024 pixels per partition row
N_TILES = ROWS // 128  # 4

_cache: dict = {}


def _split_multi_waits(nc: bass.Bass) -> int:
    """Hoist all-but-one embedded sync waits onto standalone EventSemaphore
    instructions. The walrus build behind bass2jax rejects >1 sync wait per
    instruction ("Too many sync wait commands"); a standalone wait on the
    same engine stream immediately before is semantically identical."""
    n = 0
    for fn in nc.m.functions:
        for block in fn.blocks:
            out = []
            changed = False
            for ins in block.instructions:
                si = ins.sync_info
                waits = list(si.on_wait) if si is not None else []
                if len(waits) > 1:
                    for k, w in enumerate(waits[:-1]):
                        ev = mybir.InstEventSemaphore(
                            name=f"{ins.name}-hw{k}", ins=[], outs=[]
                        )
                        ev.sync_info = bass_rust.SyncInfo(
                            on_wait=[w], on_update=[]
                        )
                        ev.engine = ins.engine
                        nc.inst_map[ev.name] = ev
                        out.append(ev)
                        n += 1
                    si.on_wait = [waits[-1]]
                    changed = True
                out.append(ins)
            if changed:
                block.instructions = out
    return n


def _build_bass() -> bass.Bass:
    f32 = mybir.dt.float32
    Alu = mybir.AluOpType
    Act = mybir.ActivationFunctionType

    nc = bass.Bass(trn_type="TRN2")
    x_d = nc.dram_tensor("x", [ROWS, FREE], f32, kind="ExternalInput")
    y_d = nc.dram_tensor("y", [ROWS, FREE], f32, kind="ExternalOutput")

    with tile.TileContext(nc) as tc:
        with (
            tc.tile_pool(name="pin", bufs=2) as pin,
            tc.tile_pool(name="pout", bufs=2) as pout,
            tc.tile_pool(name="pm", bufs=6) as pm,
            tc.tile_pool(name="ptmp", bufs=2) as ptmp,
        ):
            for i in range(N_TILES):
                xt = pin.tile([128, FREE], f32, tag="xt")
                nc.sync.dma_start(xt, x_d[i * 128 : (i + 1) * 128, :])
                ot = pout.tile([128, FREE], f32, tag="ot")

                u = ptmp.tile([128, C], f32, tag="u")
                w = ptmp.tile([128, C], f32, tag="w")
                for t in range(T):
                    # dense t-plane views (host laid the data out t-major)
                    o_t = ot[:, t * C : (t + 1) * C]
                    u_src = xt[:, 0:C] if t == 0 else u

                    m = pm.tile([128, C], f32, tag="m")
                    # m = (u <= Vth) in {0.0, 1.0}
                    nc.vector.tensor_scalar(m, u_src, VTH, None, Alu.is_le)
                    # o_t = 1 - m
                    nc.scalar.activation(o_t, m, Act.Copy, bias=1.0, scale=-1.0)
                    if t < T - 1:
                        # w = (u * TAU) * m
                        nc.vector.scalar_tensor_tensor(
                            w, u_src, TAU, m, Alu.mult, Alu.mult
                        )
                        # u = w + x_{t+1}
                        nc.vector.tensor_tensor(
                            u, w, xt[:, (t + 1) * C : (t + 2) * C], Alu.add
                        )

                nc.sync.dma_start(y_d[i * 128 : (i + 1) * 128, :], ot)

    _split_multi_waits(nc)
    return nc


def _shard(x: np.ndarray, c: int) -> np.ndarray:
    """Core c's shard, t-plane-major: [ROWS, C, T] -> [ROWS, T, C] -> flat."""
    s = x[c * B_PER_CORE : (c + 1) * B_PER_CORE].reshape(ROWS, C, T)
    return np.ascontiguousarray(s.transpose(0, 2, 1)).reshape(ROWS, FREE)


def _unshard(y: np.ndarray) -> np.ndarray:
    """Invert _shard's layout for one core's output."""
    s = y.reshape(ROWS, T, C).transpose(0, 2, 1)
    return np.ascontiguousarray(s).reshape(B_PER_CORE, *FULL_SHAPE[1:])


def kernel(x: np.ndarray) -> np.ndarray:
    assert x.shape == FULL_SHAPE, x.shape
    in_dtype = x.dtype

    if "nc" not in _cache:
        _cache["nc"] = _build_bass()
    nc = _cache["nc"]

    x = np.ascontiguousarray(x, dtype=np.float32)
    in_maps = [{"x": _shard(x, c)} for c in range(N_CORES)]
    res = run_bass_kernel_spmd(nc, in_maps, core_ids=list(range(N_CORES)))
    out = np.concatenate(
        [_unshard(res.results[c]["y"]) for c in range(N_CORES)], axis=0
    )
    return out.astype(in_dtype, copy=False)


# revision 10
# speedup vs baseline: 1.3211x; 1.1591x over previous
"""LIF spiking-neuron recurrence kernel for Trainium2 (Bass/Tile, 8-core SPMD).

Problem: x [32, 128, 32, 32, 8] f32, time on the LAST axis (T=8).
    u_0 = x_0;  o_t = (u_t > Vth);  u_{t+1} = TAU * u_t * (1 - o_t) + x_{t+1}
Output: spikes o [32, 128, 32, 32, 8] f32 (0.0 / 1.0).

Sharding: pure data-parallel over the batch dim (32 -> 4 per core, 8 cores),
no communication. While sharding, the host also lays each core's shard out
t-plane-major ([pixels, T] -> [T, pixels] per 1024-pixel row group) so every
on-chip operand is dense unit-stride; engines pay a ~2x throughput penalty on
strided (stride-8) access patterns, which the interleaved layout would force
on every timestep. The gather step inverts the layout on the way out.

Per-timestep compute (on [128, C] dense views):
    m   = (u <= Vth)                  DVE tensor_scalar (is_le), 2x f32 mode
    o_t = 1 - m                       ACT activation(Copy, scale=-1, bias=1)
    w   = (u * TAU) * m               DVE scalar_tensor_tensor (mult, mult)
    u   = w + x_{t+1}                 DVE tensor_tensor add

Multiplying by m in {0.0, 1.0} is exact, so results are bit-identical to the
reference ordering TAU*u*(1-o) + x.
"""

import numpy as np

import bass_rust
import concourse.bass as bass
import concourse.mybir as mybir
import concourse.tile as tile
from concourse.bass_utils import run_bass_kernel_spmd

VTH = 0.2
TAU = 0.25

N_CORES = 8
FULL_SHAPE = (32, 128, 32, 32, 8)
B_PER_CORE = FULL_SHAPE[0] // N_CORES  # 4
T = FULL_SHAPE[-1]  # 8

ROWS = 512  # per-core partition rows: 4*128*32*32*8 / 8192
FREE = 8192  # free dim per row
C = FREE // T  # 1024 pixels per partition row
N_TILES = ROWS // 128  # 4

_cache: dict = {}


def _split_multi_waits(nc: bass.Bass) -> int:
    """Hoist all-but-one embedded sync waits onto standalone EventSemaphore
    instructions. The walrus build behind bass2jax rejects >1 sync wait per
    instruction ("Too many sync wait commands"); a standalone wait on the
    same engine stream immediately before is semantically identical."""
    n = 0
    for fn in nc.m.functions:
        for block in fn.blocks:
            out = []
            changed = False
            for ins in block.instructions:
                si = ins.sync_info
                waits = list(si.on_wait) if si is not None else []
                if len(waits) > 1:
                    for k, w in enumerate(waits[:-1]):
                        ev = mybir.InstEventSemaphore(
                            name=f"{ins.name}-hw{k}", ins=[], outs=[]
                        )
                        ev.sync_info = bass_rust.SyncInfo(
                            on_wait=[w], on_update=[]
                        )
                        ev.engine = ins.engine
                        nc.inst_map[ev.name] = ev
                        out.append(ev)
                        n += 1
                    si.on_wait = [waits[-1]]
                    changed = True
                out.append(ins)
            if changed:
                block.instructions = out
    return n


def _build_bass() -> bass.Bass:
    f32 = mybir.dt.float32
    Alu = mybir.AluOpType
    Act = mybir.ActivationFunctionType

    nc = bass.Bass(trn_type="TRN2")
    x_d = nc.dram_tensor("x", [ROWS, FREE], f32, kind="ExternalInput")
    y_d = nc.dram_tensor("y", [ROWS, FREE], f32, kind="ExternalOutput")

    with tile.TileContext(nc) as tc:
        with (
            tc.tile_pool(name="pin", bufs=2 * T) as pin,
            tc.tile_pool(name="pout", bufs=2 * T) as pout,
            tc.tile_pool(name="pm", bufs=6) as pm,
            tc.tile_pool(name="ptmp", bufs=2) as ptmp,
        ):
            for i in range(N_TILES):
                rows = slice(i * 128, (i + 1) * 128)
                # per-t-plane loads: compute starts after plane 0 lands,
                # instead of stalling on one monolithic 4 MiB transfer
                xp = []
                for t in range(T):
                    p = pin.tile([128, C], f32, tag="xp")
                    nc.sync.dma_start(p, x_d[rows, t * C : (t + 1) * C])
                    xp.append(p)

                u = ptmp.tile([128, C], f32, tag="u")
                w = ptmp.tile([128, C], f32, tag="w")
                for t in range(T):
                    u_src = xp[0] if t == 0 else u
                    m = pm.tile([128, C], f32, tag="m")
                    o_t = pout.tile([128, C], f32, tag="op")
                    # m = (u <= Vth) in {0.0, 1.0}
                    nc.vector.tensor_scalar(m, u_src, VTH, None, Alu.is_le)
                    # o_t = 1 - m
                    nc.scalar.activation(o_t, m, Act.Copy, bias=1.0, scale=-1.0)
                    # per-plane store drains while later steps still compute
                    nc.sync.dma_start(y_d[rows, t * C : (t + 1) * C], o_t)
                    if t < T - 1:
                        # w = (u * TAU) * m
                        nc.vector.scalar_tensor_tensor(
                            w, u_src, TAU, m, Alu.mult, Alu.mult
                        )
                        # u = w + x_{t+1}
                        nc.vector.tensor_tensor(u, w, xp[t + 1], Alu.add)

    _split_multi_waits(nc)
    return nc


def _shard(x: np.ndarray, c: int) -> np.ndarray:
    """Core c's shard, t-plane-major: [ROWS, C, T] -> [ROWS, T, C] -> flat."""
    s = x[c * B_PER_CORE : (c + 1) * B_PER_CORE].reshape(ROWS, C, T)
    return np.ascontiguousarray(s.transpose(0, 2, 1)).reshape(ROWS, FREE)


def _unshard(y: np.ndarray) -> np.ndarray:
    """Invert _shard's layout for one core's output."""
    s = y.reshape(ROWS, T, C).transpose(0, 2, 1)
    return np.ascontiguousarray(s).reshape(B_PER_CORE, *FULL_SHAPE[1:])


def kernel(x: np.ndarray) -> np.ndarray:
    assert x.shape == FULL_SHAPE, x.shape
    in_dtype = x.dtype

    if "nc" not in _cache:
        _cache["nc"] = _build_bass()
    nc = _cache["nc"]

    x = np.ascontiguousarray(x, dtype=np.float32)
    in_maps = [{"x": _shard(x, c)} for c in range(N_CORES)]
    res = run_bass_kernel_spmd(nc, in_maps, core_ids=list(range(N_CORES)))
    out = np.concatenate(
        [_unshard(res.results[c]["y"]) for c in range(N_CORES)], axis=0
    )
    return out.astype(in_dtype, copy=False)


# revision 13
# speedup vs baseline: 1.3942x; 1.0553x over previous
"""LIF spiking-neuron recurrence kernel for Trainium2 (Bass/Tile, 8-core SPMD).

Problem: x [32, 128, 32, 32, 8] f32, time on the LAST axis (T=8).
    u_0 = x_0;  o_t = (u_t > Vth);  u_{t+1} = TAU * u_t * (1 - o_t) + x_{t+1}
Output: spikes o [32, 128, 32, 32, 8] f32 (0.0 / 1.0).

Sharding: pure data-parallel over the batch dim (32 -> 4 per core, 8 cores),
no communication. While sharding, the host also lays each core's shard out
t-plane-major ([pixels, T] -> [T, pixels] per 1024-pixel row group) so every
on-chip operand is dense unit-stride; engines pay a ~2x throughput penalty on
strided (stride-8) access patterns, which the interleaved layout would force
on every timestep. The gather step inverts the layout on the way out.

Per-timestep compute (on [128, C] dense views):
    m   = (u <= Vth)                  DVE tensor_scalar (is_le), 2x f32 mode
    o_t = 1 - m                       ACT activation(Copy, scale=-1, bias=1)
    w   = (u * TAU) * m               DVE scalar_tensor_tensor (mult, mult)
    u   = w + x_{t+1}                 DVE tensor_tensor add

Multiplying by m in {0.0, 1.0} is exact, so results are bit-identical to the
reference ordering TAU*u*(1-o) + x.
"""

import numpy as np

import bass_rust
import concourse.bass as bass
import concourse.mybir as mybir
import concourse.tile as tile
from concourse.bass_utils import run_bass_kernel_spmd

VTH = 0.2
TAU = 0.25

N_CORES = 8
FULL_SHAPE = (32, 128, 32, 32, 8)
B_PER_CORE = FULL_SHAPE[0] // N_CORES  # 4
T = FULL_SHAPE[-1]  # 8

ROWS = 256  # per-core partition rows: 4*128*32*32*8 / FREE
FREE = 16384  # free dim per row
C = FREE // T  # 2048 pixels per partition row
N_TILES = ROWS // 128  # 2

_cache: dict = {}


def _split_multi_waits(nc: bass.Bass) -> int:
    """Hoist all-but-one embedded sync waits onto standalone EventSemaphore
    instructions. The walrus build behind bass2jax rejects >1 sync wait per
    instruction ("Too many sync wait commands"); a standalone wait on the
    same engine stream immediately before is semantically identical."""
    n = 0
    for fn in nc.m.functions:
        for block in fn.blocks:
            out = []
            changed = False
            for ins in block.instructions:
                si = ins.sync_info
                waits = list(si.on_wait) if si is not None else []
                if len(waits) > 1:
                    for k, w in enumerate(waits[:-1]):
                        ev = mybir.InstEventSemaphore(
                            name=f"{ins.name}-hw{k}", ins=[], outs=[]
                        )
                        ev.sync_info = bass_rust.SyncInfo(
                            on_wait=[w], on_update=[]
                        )
                        ev.engine = ins.engine
                        nc.inst_map[ev.name] = ev
                        out.append(ev)
                        n += 1
                    si.on_wait = [waits[-1]]
                    changed = True
                out.append(ins)
            if changed:
                block.instructions = out
    return n


def _build_bass() -> bass.Bass:
    f32 = mybir.dt.float32
    Alu = mybir.AluOpType
    Act = mybir.ActivationFunctionType

    nc = bass.Bass(trn_type="TRN2")
    x_d = nc.dram_tensor("x", [ROWS, FREE], f32, kind="ExternalInput")
    y_d = nc.dram_tensor("y", [ROWS, FREE], f32, kind="ExternalOutput")

    with tile.TileContext(nc) as tc:
        with (
            tc.tile_pool(name="pin", bufs=10) as pin,
            tc.tile_pool(name="pout", bufs=6) as pout,
            tc.tile_pool(name="pm", bufs=3) as pm,
            tc.tile_pool(name="ptmp", bufs=2) as ptmp,
        ):
            for i in range(N_TILES):
                rows = slice(i * 128, (i + 1) * 128)
                # per-t-plane loads: compute starts after plane 0 lands,
                # instead of stalling on one monolithic 4 MiB transfer
                xp = []
                for t in range(T):
                    p = pin.tile([128, C], f32, tag="xp")
                    nc.sync.dma_start(p, x_d[rows, t * C : (t + 1) * C])
                    xp.append(p)

                u = ptmp.tile([128, C], f32, tag="u")
                w = ptmp.tile([128, C], f32, tag="w")
                for t in range(T):
                    u_src = xp[0] if t == 0 else u
                    m = pm.tile([128, C], f32, tag="m")
                    o_t = pout.tile([128, C], f32, tag="op")
                    # m = (u <= Vth) in {0.0, 1.0}
                    nc.vector.tensor_scalar(m, u_src, VTH, None, Alu.is_le)
                    # o_t = 1 - m
                    nc.scalar.activation(o_t, m, Act.Copy, bias=1.0, scale=-1.0)
                    # per-plane store drains while later steps still compute;
                    # issued from ACT (also HWDGE) so SP's issue queue — which
                    # serializes at ~0.6us per dma_start — only carries loads
                    nc.scalar.dma_start(y_d[rows, t * C : (t + 1) * C], o_t)
                    if t < T - 1:
                        # w = (u * TAU) * m
                        nc.vector.scalar_tensor_tensor(
                            w, u_src, TAU, m, Alu.mult, Alu.mult
                        )
                        # u = w + x_{t+1}
                        nc.vector.tensor_tensor(u, w, xp[t + 1], Alu.add)

    _split_multi_waits(nc)
    return nc


def _shard(x: np.ndarray, c: int) -> np.ndarray:
    """Core c's shard, t-plane-major: [ROWS, C, T] -> [ROWS, T, C] -> flat."""
    s = x[c * B_PER_CORE : (c + 1) * B_PER_CORE].reshape(ROWS, C, T)
    return np.ascontiguousarray(s.transpose(0, 2, 1)).reshape(ROWS, FREE)


def _unshard(y: np.ndarray) -> np.ndarray:
    """Invert _shard's layout for one core's output."""
    s = y.reshape(ROWS, T, C).transpose(0, 2, 1)
    return np.ascontiguousarray(s).reshape(B_PER_CORE, *FULL_SHAPE[1:])


def kernel(x: np.ndarray) -> np.ndarray:
    assert x.shape == FULL_SHAPE, x.shape
    in_dtype = x.dtype

    if "nc" not in _cache:
        _cache["nc"] = _build_bass()
    nc = _cache["nc"]

    x = np.ascontiguousarray(x, dtype=np.float32)
    in_maps = [{"x": _shard(x, c)} for c in range(N_CORES)]
    res = run_bass_kernel_spmd(nc, in_maps, core_ids=list(range(N_CORES)))
    out = np.concatenate(
        [_unshard(res.results[c]["y"]) for c in range(N_CORES)], axis=0
    )
    return out.astype(in_dtype, copy=False)


# revision 15
# speedup vs baseline: 1.5111x; 1.0839x over previous
"""LIF spiking-neuron recurrence kernel for Trainium2 (Bass/Tile, 8-core SPMD).

Problem: x [32, 128, 32, 32, 8] f32, time on the LAST axis (T=8).
    u_0 = x_0;  o_t = (u_t > Vth);  u_{t+1} = TAU * u_t * (1 - o_t) + x_{t+1}
Output: spikes o [32, 128, 32, 32, 8] f32 (0.0 / 1.0).

Sharding: pure data-parallel over the batch dim (32 -> 4 per core, 8 cores),
no communication. While sharding, the host also lays each core's shard out
t-plane-major ([pixels, T] -> [T, pixels] per 1024-pixel row group) so every
on-chip operand is dense unit-stride; engines pay a ~2x throughput penalty on
strided (stride-8) access patterns, which the interleaved layout would force
on every timestep. The gather step inverts the layout on the way out.

Per-timestep compute (on [128, C] dense views):
    m   = (u <= Vth)                  DVE tensor_scalar (is_le), 2x f32 mode
    o_t = 1 - m                       ACT activation(Copy, scale=-1, bias=1)
    w   = (u * TAU) * m               DVE scalar_tensor_tensor (mult, mult)
    u   = w + x_{t+1}                 DVE tensor_tensor add

Multiplying by m in {0.0, 1.0} is exact, so results are bit-identical to the
reference ordering TAU*u*(1-o) + x.
"""

import numpy as np

import bass_rust
import concourse.bass as bass
import concourse.mybir as mybir
import concourse.tile as tile
from concourse.bass_utils import run_bass_kernel_spmd

VTH = 0.2
TAU = 0.25

N_CORES = 8
FULL_SHAPE = (32, 128, 32, 32, 8)
B_PER_CORE = FULL_SHAPE[0] // N_CORES  # 4
T = FULL_SHAPE[-1]  # 8

ROWS = 256  # per-core partition rows: 4*128*32*32*8 / FREE
FREE = 16384  # free dim per row
C = FREE // T  # 2048 pixels per partition row
N_TILES = ROWS // 128  # 2

_cache: dict = {}


def _split_multi_waits(nc: bass.Bass) -> int:
    """Hoist all-but-one embedded sync waits onto standalone EventSemaphore
    instructions. The walrus build behind bass2jax rejects >1 sync wait per
    instruction ("Too many sync wait commands"); a standalone wait on the
    same engine stream immediately before is semantically identical."""
    n = 0
    for fn in nc.m.functions:
        for block in fn.blocks:
            out = []
            changed = False
            for ins in block.instructions:
                si = ins.sync_info
                waits = list(si.on_wait) if si is not None else []
                if len(waits) > 1:
                    for k, w in enumerate(waits[:-1]):
                        ev = mybir.InstEventSemaphore(
                            name=f"{ins.name}-hw{k}", ins=[], outs=[]
                        )
                        ev.sync_info = bass_rust.SyncInfo(
                            on_wait=[w], on_update=[]
                        )
                        ev.engine = ins.engine
                        nc.inst_map[ev.name] = ev
                        out.append(ev)
                        n += 1
                    si.on_wait = [waits[-1]]
                    changed = True
                out.append(ins)
            if changed:
                block.instructions = out
    return n


def _build_bass() -> bass.Bass:
    f32 = mybir.dt.float32
    Alu = mybir.AluOpType
    Act = mybir.ActivationFunctionType

    nc = bass.Bass(trn_type="TRN2")
    x_d = nc.dram_tensor("x", [ROWS, FREE], f32, kind="ExternalInput")
    y_d = nc.dram_tensor("y", [ROWS, FREE], f32, kind="ExternalOutput")

    with tile.TileContext(nc) as tc:
        with (
            tc.tile_pool(name="pin", bufs=12) as pin,
            tc.tile_pool(name="pout", bufs=4) as pout,
            tc.tile_pool(name="pm", bufs=3) as pm,
            tc.tile_pool(name="ptmp", bufs=2) as ptmp,
        ):
            for i in range(N_TILES):
                rows = slice(i * 128, (i + 1) * 128)
                # per-t-plane loads: compute starts after plane 0 lands,
                # instead of stalling on one monolithic 4 MiB transfer
                xp = []
                for t in range(T):
                    p = pin.tile([128, C], f32, tag="xp")
                    nc.sync.dma_start(p, x_d[rows, t * C : (t + 1) * C])
                    xp.append(p)

                u = ptmp.tile([128, C], f32, tag="u")
                w = ptmp.tile([128, C], f32, tag="w")
                for t in range(T - 1):
                    u_src = xp[0] if t == 0 else u
                    m = pm.tile([128, C], f32, tag="m")
                    o_t = pout.tile([128, C], f32, tag="op")
                    # m = (u <= Vth) in {0.0, 1.0}
                    nc.vector.tensor_scalar(m, u_src, VTH, None, Alu.is_le)
                    # o_t = 1 - m
                    nc.scalar.activation(o_t, m, Act.Copy, bias=1.0, scale=-1.0)
                    # per-plane store drains while later steps still compute;
                    # issued from ACT (also HWDGE) so SP's issue queue — which
                    # serializes at ~0.6us per dma_start — only carries loads
                    nc.scalar.dma_start(y_d[rows, t * C : (t + 1) * C], o_t)
                    # w = (u * TAU) * m
                    nc.vector.scalar_tensor_tensor(
                        w, u_src, TAU, m, Alu.mult, Alu.mult
                    )
                    # u = w + x_{t+1}
                    nc.vector.tensor_tensor(u, w, xp[t + 1], Alu.add)

                # t = T-1: no state update needed, so skip m/ACT and emit
                # o = (u > Vth) straight from DVE in two half-planes whose
                # stores overlap — keeps the kernel tail short
                H = C // 2
                for h in range(2):
                    o_t = pout.tile([128, H], f32, tag="oh")
                    cols = slice(h * H, (h + 1) * H)
                    nc.vector.tensor_scalar(
                        o_t, u[:, cols], VTH, None, Alu.is_gt
                    )
                    nc.sync.dma_start(
                        y_d[rows, (T - 1) * C + h * H : (T - 1) * C + (h + 1) * H],
                        o_t,
                    )

    _split_multi_waits(nc)
    return nc


def _shard(x: np.ndarray, c: int) -> np.ndarray:
    """Core c's shard, t-plane-major: [ROWS, C, T] -> [ROWS, T, C] -> flat."""
    s = x[c * B_PER_CORE : (c + 1) * B_PER_CORE].reshape(ROWS, C, T)
    return np.ascontiguousarray(s.transpose(0, 2, 1)).reshape(ROWS, FREE)


def _unshard(y: np.ndarray) -> np.ndarray:
    """Invert _shard's layout for one core's output."""
    s = y.reshape(ROWS, T, C).transpose(0, 2, 1)
    return np.ascontiguousarray(s).reshape(B_PER_CORE, *FULL_SHAPE[1:])


def kernel(x: np.ndarray) -> np.ndarray:
    assert x.shape == FULL_SHAPE, x.shape
    in_dtype = x.dtype

    if "nc" not in _cache:
        _cache["nc"] = _build_bass()
    nc = _cache["nc"]

    x = np.ascontiguousarray(x, dtype=np.float32)
    in_maps = [{"x": _shard(x, c)} for c in range(N_CORES)]
    res = run_bass_kernel_spmd(nc, in_maps, core_ids=list(range(N_CORES)))
    out = np.concatenate(
        [_unshard(res.results[c]["y"]) for c in range(N_CORES)], axis=0
    )
    return out.astype(in_dtype, copy=False)
